# revision 2
# baseline (speedup 1.0000x reference)
"""Trainium2 Bass kernel for nn_MultiHeadAttention (B=2, S=2048, E=1024, H=8, D=128).

Sharding (8 cores): core c handles batch b=c//4 and head-pair g=c%4
(heads 2g, 2g+1 -> E-columns [256g, 256g+256)).
 - Q/K/V projections column-parallel (each core computes its 256 columns).
 - Attention device-local per head, computed in transposed score layout
   scoresT[k, q] so softmaxed weights are directly the rhs of attn@V.
 - Out-projection row-parallel: each core produces a full-shape partial
   out_partial = attn_out_heads @ Wo[rows]; host sums 4 partials per batch.
 - Causal structure: fully-masked (strictly upper) 128x512 blocks are skipped;
   diagonal-straddling blocks apply the actual mask values (additively,
   pre-exp) via identity matmuls.
"""

import os
import sys

for _p in ("/opt/trn_rl_repo", os.environ.get("TRN_RL_REPO", "")):
    if _p and os.path.isdir(_p) and _p not in sys.path:
        sys.path.insert(0, _p)

import numpy as np
import ml_dtypes

BF16 = ml_dtypes.bfloat16

B, S, E, H = 2, 2048, 1024, 8
D = E // H          # 128
HP = 2              # heads per core
C = HP * D          # 256 projection columns per core
NCORES = 8
KT = S // 128       # 16 k-tiles
QC = S // 512       # 4 q-chunks
SCALE = 1.0 / float(np.sqrt(D))
MASK_NEG = -30000.0

_prog_cache = {}


def build_program(n_iters: int = 1):
    """Build the SPMD Bass program (Tile). Returns the compiled Bacc object."""
    import concourse.bass as bass
    import concourse.mybir as mybir
    import concourse.tile as tile
    from concourse import bacc
    from concourse.masks import make_identity
    from contextlib import ExitStack

    f32 = mybir.dt.float32
    bf16 = mybir.dt.bfloat16
    AF = mybir.ActivationFunctionType

    nc = bacc.Bacc("TRN2", target_bir_lowering=False, debug=False,
                   enable_partition_id=False)

    # ---- DRAM I/O (per-core slices supplied by the host) ----
    xq_t = nc.dram_tensor("xq_t", [E, S], bf16, kind="ExternalInput")
    xk_t = nc.dram_tensor("xk_t", [E, S], bf16, kind="ExternalInput")
    xv_t = nc.dram_tensor("xv_t", [E, S], bf16, kind="ExternalInput")
    wq_d = nc.dram_tensor("wq", [E, C], bf16, kind="ExternalInput")
    wk_d = nc.dram_tensor("wk", [E, C], bf16, kind="ExternalInput")
    wv_d = nc.dram_tensor("wv", [E, C], bf16, kind="ExternalInput")
    wo_d = nc.dram_tensor("wo", [C, E], bf16, kind="ExternalInput")
    bqk_d = nc.dram_tensor("bqk", [128, 4], f32, kind="ExternalInput")
    bv_d = nc.dram_tensor("bv_bc", [128, C], f32, kind="ExternalInput")
    bo_d = nc.dram_tensor("bo_bc", [128, E], f32, kind="ExternalInput")
    maskt_d = nc.dram_tensor("maskt", [KT, 128, 512], bf16, kind="ExternalInput")
    out_d = nc.dram_tensor("out", [S, E], f32, kind="ExternalOutput")

    with tile.TileContext(nc) as tc, ExitStack() as ctx:
        persist = ctx.enter_context(tc.tile_pool(name="persist", bufs=1))
        xt_pool = ctx.enter_context(tc.tile_pool(name="xt", bufs=2))
        expt_pool = ctx.enter_context(tc.tile_pool(name="expt", bufs=18))
        acc_pool = ctx.enter_context(tc.tile_pool(name="acc", bufs=2))
        outst = ctx.enter_context(tc.tile_pool(name="outst", bufs=3))

        # ---- constants ----
        ident = persist.tile([128, 128], bf16, tag="ident")
        make_identity(nc, ident)
        ones_col = persist.tile([128, 1], f32, tag="ones_col")
        nc.vector.memset(ones_col, 1.0)
        ones_row = persist.tile([1, 128], f32, tag="ones_row")
        nc.vector.memset(ones_row, 1.0)

        # ---- persistent weight / bias / mask tiles ----
        wq_sb = persist.tile([128, 8, C], bf16, tag="wq")
        wk_sb = persist.tile([128, 8, C], bf16, tag="wk")
        wv_sb = persist.tile([128, 8, C], bf16, tag="wv")
        wo_sb = persist.tile([128, HP, E], bf16, tag="wo")
        bqk = persist.tile([128, 4], f32, tag="bqk")
        bv_bc = persist.tile([128, C], f32, tag="bv")
        bo_bc = persist.tile([128, E], f32, tag="bo")
        maskt_sb = persist.tile([128, KT, 512], bf16, tag="maskt")

        nc.sync.dma_start(out=wq_sb, in_=wq_d.ap().rearrange("(c p) n -> p c n", p=128))
        nc.sync.dma_start(out=wk_sb, in_=wk_d.ap().rearrange("(c p) n -> p c n", p=128))
        nc.sync.dma_start(out=wv_sb, in_=wv_d.ap().rearrange("(c p) n -> p c n", p=128))
        nc.sync.dma_start(out=wo_sb, in_=wo_d.ap().rearrange("(h p) n -> p h n", p=128))
        nc.sync.dma_start(out=bqk, in_=bqk_d.ap())
        nc.sync.dma_start(out=bv_bc, in_=bv_d.ap())
        nc.sync.dma_start(out=bo_bc, in_=bo_d.ap())
        nc.sync.dma_start(out=maskt_sb, in_=maskt_d.ap().rearrange("c p n -> p c n"))

        for _ in range(n_iters):
            # per-head persistent activations
            qt_sb = [persist.tile([128, S], bf16, tag=f"qt{m}", name=f"qt{m}") for m in range(HP)]
            kt_sb = [persist.tile([128, S], bf16, tag=f"kt{m}", name=f"kt{m}") for m in range(HP)]
            v_sb = persist.tile([128, KT, C], bf16, tag="v", name="v")
            ot_sb = [persist.tile([128, S], bf16, tag=f"ot{m}", name=f"ot{m}") for m in range(HP)]

            # ================= Phase 1: projections =================
            with tc.tile_pool(name="ps_proj", bufs=3, space="PSUM") as ps_proj:
                # QT / KT: [C, S] = W.T @ X.T  (lhsT = W chunk, rhs = XT chunk)
                for name, xdram, wsb, qkts, bcol in (
                    ("q", xq_t, wq_sb, qt_sb, 0),
                    ("k", xk_t, wk_sb, kt_sb, 2),
                ):
                    xt = xt_pool.tile([128, 8, S], bf16, tag="xt")
                    nc.sync.dma_start(
                        out=xt, in_=xdram.ap().rearrange("(c p) n -> p c n", p=128))
                    for m in range(HP):
                        for n in range(QC):
                            ps = ps_proj.tile([128, 512], f32, tag="ps_proj")
                            for c in range(8):
                                nc.tensor.matmul(
                                    ps,
                                    lhsT=wsb[:, c, m * 128:(m + 1) * 128],
                                    rhs=xt[:, c, n * 512:(n + 1) * 512],
                                    start=(c == 0), stop=(c == 7))
                            nc.scalar.activation(
                                out=qkts[m][:, n * 512:(n + 1) * 512], in_=ps,
                                func=AF.Identity,
                                bias=bqk[:, bcol + m:bcol + m + 1], scale=1.0)

                # V natural: [S, C] = X @ Wv (lhsT = XT chunk slice, rhs = Wv chunk)
                xt = xt_pool.tile([128, 8, S], bf16, tag="xt")
                nc.sync.dma_start(
                    out=xt, in_=xv_t.ap().rearrange("(c p) n -> p c n", p=128))
                for s in range(KT):
                    ps = ps_proj.tile([128, C], f32, tag="ps_proj")
                    for c in range(8):
                        nc.tensor.matmul(
                            ps,
                            lhsT=xt[:, c, s * 128:(s + 1) * 128],
                            rhs=wv_sb[:, c, :],
                            start=(c == 0), stop=(c == 7))
                    nc.vector.tensor_add(v_sb[:, s, :], ps, bv_bc)

            # ================= Phase 2: attention (per head) =================
            with tc.tile_pool(name="ps_sc", bufs=2, space="PSUM") as ps_sc, \
                 tc.tile_pool(name="ps_ot", bufs=2, space="PSUM") as ps_ot, \
                 tc.tile_pool(name="ps_cs", bufs=2, space="PSUM") as ps_cs, \
                 tc.tile_pool(name="ps_rs", bufs=2, space="PSUM") as ps_rs:
                for h in range(HP):
                    cseng = nc.vector if h == 0 else nc.gpsimd
                    for j in range(QC):
                        nk = 4 * (j + 1)
                        qsl = slice(j * 512, (j + 1) * 512)
                        ets = []
                        for kti in range(nk):
                            ps = ps_sc.tile([128, 512], f32, tag="ps_sc")
                            diag = kti >= 4 * j
                            if diag:
                                nc.tensor.matmul(ps, lhsT=ident,
                                                 rhs=maskt_sb[:, kti, :],
                                                 start=True, stop=False)
                            nc.tensor.matmul(
                                ps,
                                lhsT=kt_sb[h][:, kti * 128:(kti + 1) * 128],
                                rhs=qt_sb[h][:, qsl],
                                start=(not diag), stop=True)
                            et = expt_pool.tile([128, 512], bf16, tag="et")
                            nc.scalar.activation(out=et, in_=ps, func=AF.Exp,
                                                 scale=SCALE)
                            ets.append(et)

                        # attn @ V -> outT[d, qchunk] (accumulate over k-tiles)
                        ot = ps_ot.tile([128, 512], f32, tag="ps_ot")
                        for kti in range(nk):
                            nc.tensor.matmul(
                                ot,
                                lhsT=v_sb[:, kti, h * 128:(h + 1) * 128],
                                rhs=ets[kti],
                                start=(kti == 0), stop=(kti == nk - 1))

                        # column sums -> 1/colsum broadcast
                        accum = acc_pool.tile([128, 512], f32, tag="accum")
                        cseng.tensor_add(accum, ets[0], ets[1])
                        for i in range(2, nk):
                            cseng.tensor_add(accum, accum, ets[i])
                        cs = ps_cs.tile([1, 512], f32, tag="ps_cs")
                        nc.tensor.matmul(cs, lhsT=ones_col, rhs=accum,
                                         start=True, stop=True)
                        rinv = acc_pool.tile([1, 512], f32, tag="rinv")
                        nc.vector.reciprocal(rinv, cs)
                        rs_ps = ps_rs.tile([128, 512], f32, tag="ps_rs")
                        nc.tensor.matmul(rs_ps, lhsT=ones_row, rhs=rinv,
                                         start=True, stop=True)
                        rs_sb = acc_pool.tile([128, 512], f32, tag="rssb")
                        nc.scalar.copy(rs_sb, rs_ps)

                        # normalize
                        nc.vector.tensor_mul(ot_sb[h][:, qsl], ot, rs_sb)

            # ================= Phase 3: out-projection =================
            with tc.tile_pool(name="ps_op", bufs=3, space="PSUM") as ps_op:
                for s in range(KT):
                    osb = outst.tile([128, E], f32, tag="osb")
                    for nch in range(2):
                        nsl = slice(nch * 512, (nch + 1) * 512)
                        ps = ps_op.tile([128, 512], f32, tag="ps_op")
                        for h in range(HP):
                            nc.tensor.matmul(
                                ps,
                                lhsT=ot_sb[h][:, s * 128:(s + 1) * 128],
                                rhs=wo_sb[:, h, nsl],
                                start=(h == 0), stop=(h == HP - 1))
                        nc.vector.tensor_add(osb[:, nsl], ps, bo_bc[:, nsl])
                    nc.gpsimd.dma_start(out=out_d[s * 128:(s + 1) * 128, :], in_=osb)

    nc.compile()
    return nc


def get_program(n_iters: int = 1):
    if n_iters not in _prog_cache:
        _prog_cache[n_iters] = build_program(n_iters)
    return _prog_cache[n_iters]


def make_in_maps(query, key_, value, Wq, bq, Wk, bk, Wv, bv, Wo, bo, mask):
    """Host-side sharding: build the 8 per-core input maps."""
    query = np.asarray(query, np.float32)
    key_ = np.asarray(key_, np.float32)
    value = np.asarray(value, np.float32)
    mask = np.asarray(mask)

    # transposed bf16 activations per batch: [E, S]
    xt = {}
    for b in range(B):
        xt[("q", b)] = np.ascontiguousarray(
            query[b].T.astype(BF16))
        xt[("k", b)] = np.ascontiguousarray(key_[b].T.astype(BF16))
        xt[("v", b)] = np.ascontiguousarray(value[b].T.astype(BF16))

    # additive transposed mask, diagonal 128x512 blocks only
    m2 = np.asarray(mask).reshape(S, S)
    maskt = np.empty((KT, 128, 512), np.float32)
    for j in range(QC):
        q0 = j * 512
        blk = m2[q0:q0 + 512, q0:q0 + 512]           # [q, k]
        add = np.where(blk.T != 0, 0.0, MASK_NEG)    # [k, q]
        # note: additive mask is applied pre-scale, so divide by SCALE
        add = add / SCALE
        for i in range(4):
            maskt[4 * j + i] = add[i * 128:(i + 1) * 128, :]
    maskt = maskt.astype(BF16)

    Wq = np.asarray(Wq, np.float32)
    Wk = np.asarray(Wk, np.float32)
    Wv = np.asarray(Wv, np.float32)
    Wo = np.asarray(Wo, np.float32)
    bq = np.asarray(bq, np.float32)
    bk = np.asarray(bk, np.float32)
    bv = np.asarray(bv, np.float32)
    bo = np.asarray(bo, np.float32)

    in_maps = []
    for c in range(NCORES):
        b, g = divmod(c, 4)
        c0 = C * g
        bqk = np.stack([bq[c0:c0 + 128], bq[c0 + 128:c0 + 256],
                        bk[c0:c0 + 128], bk[c0 + 128:c0 + 256]], axis=1)
        in_maps.append({
            "xq_t": xt[("q", b)],
            "xk_t": xt[("k", b)],
            "xv_t": xt[("v", b)],
            "wq": Wq[:, c0:c0 + C].astype(BF16),
            "wk": Wk[:, c0:c0 + C].astype(BF16),
            "wv": Wv[:, c0:c0 + C].astype(BF16),
            "wo": np.ascontiguousarray(Wo[c0:c0 + C, :]).astype(BF16),
            "bqk": np.ascontiguousarray(bqk, dtype=np.float32),
            "bv_bc": np.broadcast_to(bv[c0:c0 + C], (128, C)).astype(np.float32),
            "bo_bc": (np.broadcast_to(bo, (128, E)).astype(np.float32)
                      if g == 0 else np.zeros((128, E), np.float32)),
            "maskt": maskt,
        })
    return in_maps


def gather_output(results):
    out = np.zeros((B, S, E), np.float32)
    for c in range(NCORES):
        b = c // 4
        out[b] += results[c]["out"]
    return out


def kernel(**inputs) -> np.ndarray:
    from concourse.bass_utils import run_bass_kernel_spmd

    nc = get_program(1)
    in_maps = make_in_maps(**inputs)
    res = run_bass_kernel_spmd(nc, in_maps, core_ids=list(range(NCORES)))
    return gather_output(res.results)
